# revision 2
# baseline (speedup 1.0000x reference)
"""Biased MF batch scoring on 8 NeuronCores — tuned indirect-gather kernel.

Same combined-table structure as the 58us baseline (32 single-offset
indirect_dma_start ops on the Pool SWDGE; multi-offset ops are broken on HW:
they stream a contiguous run from the first offset), with:
  - fp16 table rows (132B): halves descriptor payload (drain time) + 2x DVE.
  - uneven chunks [8,8,8,6,2]: the last compute chunk covers 2 ops (128
    elements) so the tail (last DMA completion + mul + reduce + store) is
    small.
  - output stores split: cols for chunks 0-3 store while chunk 4 computes.

Layout: idx column k in [0,32): op k gathers row idx[p, k] into
rows[:, k*W:(k+1)*W].  Columns 2c*G..: chunk c has CH[c] ops: first half user
rows, second half item rows of the chunk's elements.
user row = [uf(64)|ub|1], item row = [itf(64)|1|ib+3.5] (fp16, W=66): the
row-pair dot is the final answer.
"""

import numpy as np

GLOBAL_AVERAGE = 3.5
NUM_USERS = 1_000_000
NUM_ITEMS = 100_000
F = 64
B = 16384
NCORES = 8
BC = B // NCORES
P = 128
EPP = BC // P  # 16 elements per partition
W = F + 2  # 66
CH = [8, 8, 8, 6, 2]  # ops per chunk (each chunk: CH/2 user ops + CH/2 item)
NCH = len(CH)

TRACE = False
LAST_RES = None
_BUILD_CACHE = {}


def build_nc():
    if 0 in _BUILD_CACHE:
        return _BUILD_CACHE[0]
    import concourse.bass as bass
    import concourse.mybir as mybir
    from concourse.bass import IndirectOffsetOnAxis
    from contextlib import ExitStack

    ncat = NUM_USERS + NUM_ITEMS
    nc = bass.Bass()
    idx = nc.dram_tensor("idx", [P, 2 * EPP], mybir.dt.int32, kind="ExternalInput")
    cat = nc.dram_tensor("cat", [ncat, W], mybir.dt.float16, kind="ExternalInput")
    out = nc.dram_tensor("out", [P, EPP], mybir.dt.float32, kind="ExternalOutput")

    with ExitStack() as stack:
        e = stack.enter_context
        t_idx = e(nc.sbuf_tensor("t_idx", [P, 2 * EPP], mybir.dt.int32))
        rows = e(nc.sbuf_tensor("rows", [P, 2 * EPP * W], mybir.dt.float16))
        prod = e(nc.sbuf_tensor("prod", [P, EPP * W], mybir.dt.float16))
        res = e(nc.sbuf_tensor("res", [P, EPP], mybir.dt.float32))
        s_idx = e(nc.semaphore("s_idx"))
        s_g = [e(nc.semaphore(f"s_g{c}")) for c in range(NCH)]
        s_v = e(nc.semaphore("s_v"))
        s_c = e(nc.semaphore("s_c"))
        s_o = e(nc.semaphore("s_o"))
        block = e(nc.Block())

        # chunk -> (op offset, element-column offset, #element-cols)
        op_off = [sum(CH[:c]) for c in range(NCH)]
        ecols = [ch // 2 for ch in CH]
        ecol_off = [sum(ecols[:c]) for c in range(NCH)]

        @block.sync
        def _(sy):
            sy.dma_start(t_idx[:], idx[:]).then_inc(s_idx, 16)
            sy.wait_ge(s_c, NCH - 1)
            sy.dma_start(
                out[:, : ecol_off[NCH - 1]], res[:, : ecol_off[NCH - 1]]
            ).then_inc(s_o, 16)
            sy.wait_ge(s_c, NCH)
            with nc.allow_non_contiguous_dma(reason="single trailing column"):
                sy.dma_start(
                    out[:, ecol_off[NCH - 1] :], res[:, ecol_off[NCH - 1] :]
                ).then_inc(s_o, 16)
            sy.wait_ge(s_o, 32)

        @block.gpsimd
        def _(g):
            g.wait_ge(s_idx, 16)
            for c in range(NCH):
                for j in range(CH[c]):
                    k = op_off[c] + j
                    g.indirect_dma_start(
                        out=rows[:, k * W : (k + 1) * W],
                        out_offset=None,
                        in_=cat[:],
                        in_offset=IndirectOffsetOnAxis(
                            ap=t_idx[:, k : k + 1], axis=0
                        ),
                    ).then_inc(s_g[c], 16)

        @block.vector
        def _(vec):
            for c in range(NCH):
                h = CH[c] // 2
                lo = op_off[c] * W
                po = ecol_off[c] * W
                vec.wait_ge(s_g[c], 16 * CH[c])
                vec.tensor_mul(
                    prod[:, po : po + h * W],
                    rows[:, lo : lo + h * W],
                    rows[:, lo + h * W : lo + 2 * h * W],
                ).then_inc(s_v, 1)
                vec.wait_ge(s_v, c + 1)
                vec.reduce_sum(
                    res[:, ecol_off[c] : ecol_off[c] + h],
                    prod[:, po : po + h * W].rearrange("p (g w) -> p g w", w=W),
                    axis=mybir.AxisListType.X,
                ).then_inc(s_c, 1)

    nc.finalize()
    _strip_boot_barrier(nc)
    _BUILD_CACHE[0] = nc
    return nc


def _strip_boot_barrier(nc):
    barrier_sem_ids = set()
    for bb in nc.m.functions[0].blocks:
        for ins in bb.instructions:
            si = ins.sync_info
            if si:
                for u in list(si.on_update or []) + list(si.on_wait or []):
                    if "barrier_" in (getattr(u, "ant_name", "") or ""):
                        barrier_sem_ids.add(u.id)
    for bb in nc.m.functions[0].blocks:
        if bb.name != "main":
            continue
        keep = []
        for ins in bb.instructions:
            tn = type(ins).__name__
            drop = tn == "InstMemset"
            si = ins.sync_info
            if not drop and si and tn in ("InstDrain", "InstEventSemaphore"):
                drop = any(
                    getattr(u, "id", None) in barrier_sem_ids
                    for u in list(si.on_update or []) + list(si.on_wait or [])
                )
            if not drop:
                keep.append(ins)
        if len(keep) != len(bb.instructions):
            bb.instructions[:] = keep
    used = set()
    for bb in nc.m.functions[0].blocks:
        for ins in bb.instructions:
            si = ins.sync_info
            if si:
                for u in list(si.on_update or []) + list(si.on_wait or []):
                    sid = getattr(u, "id", None)
                    if sid is not None:
                        used.add(sid)
    for bb in nc.m.functions[0].blocks:
        keep = []
        for ins in bb.instructions:
            drop = False
            if type(ins).__name__ == "InstEventSemaphore":
                si = ins.sync_info
                ups = list(si.on_update or []) if si else []
                ws = list(si.on_wait or []) if si else []
                if not ws and len(ups) == 1:
                    u = ups[0]
                    if (
                        getattr(u, "value", None) == 0
                        and getattr(u, "sem_op", None) in ("set", "assign", None)
                        and getattr(u, "id", -1) not in used
                    ):
                        drop = True
            if not drop:
                keep.append(ins)
        if len(keep) != len(bb.instructions):
            bb.instructions[:] = keep


def make_cat(user_factors, item_factors, user_biases, item_biases):
    nu, f = user_factors.shape
    ni = item_factors.shape[0]
    cat = np.empty((nu + ni, W), np.float16)
    cat[:nu, :f] = user_factors
    cat[:nu, f] = np.asarray(user_biases).reshape(nu)
    cat[:nu, f + 1] = 1.0
    cat[nu:, :f] = item_factors
    cat[nu:, f] = 1.0
    cat[nu:, f + 1] = np.asarray(item_biases).reshape(ni) + np.float32(GLOBAL_AVERAGE)
    return cat


def make_idx(users, items):
    """Element (core, p, e) = batch index core*BC + e*P + p, e in [0,16).
    Chunk c covers elements e in [ecol_off[c], ecol_off[c]+CH[c]//2); its user
    ops are idx columns op_off[c]..+CH[c]//2-1 (element order), item ops next.
    """
    u = np.asarray(users, dtype=np.int32).reshape(NCORES, EPP, P)
    it = np.asarray(items, dtype=np.int32).reshape(NCORES, EPP, P) + np.int32(NUM_USERS)
    out = np.empty((NCORES, P, 2 * EPP), np.int32)
    op_off = [sum(CH[:c]) for c in range(NCH)]
    ecols = [ch // 2 for ch in CH]
    ecol_off = [sum(ecols[:c]) for c in range(NCH)]
    for c in range(NCH):
        h = ecols[c]
        esl = slice(ecol_off[c], ecol_off[c] + h)
        out[:, :, op_off[c] : op_off[c] + h] = u[:, esl, :].transpose(0, 2, 1)
        out[:, :, op_off[c] + h : op_off[c] + 2 * h] = it[:, esl, :].transpose(0, 2, 1)
    return out


def kernel(users, items, user_factors, item_factors, user_biases, item_biases):
    global LAST_RES
    from concourse.bass_utils import run_bass_kernel_spmd

    nc = build_nc()
    cat = make_cat(user_factors, item_factors, user_biases, item_biases)
    idx = make_idx(users, items)
    in_maps = [{"idx": idx[c], "cat": cat} for c in range(NCORES)]
    res = run_bass_kernel_spmd(nc, in_maps, core_ids=list(range(NCORES)), trace=TRACE)
    LAST_RES = res
    outs = []
    for c in range(NCORES):
        o = res.results[c]["out"]  # [P, EPP]; element (p, e) = c*BC + e*P + p
        outs.append(o.T.reshape(-1))
    return np.concatenate(outs).astype(np.float32)
